# revision 7
# baseline (speedup 1.0000x reference)
"""Trainium2 Bass kernel for ClippedRelativeSelfAttention.

Sharding: 8 cores = 2 batch-groups x 4 head-groups. Each core computes
4 batches x 2 heads and a partial output projection; host sums partials.

Math notes (vs reference):
 - softmax_kernel's `ratio` multiplier cancels exactly in the final output
   (numerator and denominator are both linear in qf and in rf), so we drop it.
 - q_rel[l,k] = sum_r qdot[l,r] * MASK_r[l,k]  (MASK_r = [DIFFS==r], r<4);
   DIFFS==4 entries contribute 0 because rfm[4] = rf[4]-mde = 0.
 - denominator: qf . denom = sum_{r<4} cnt[l,r]*qdot[l,r] + L*qmd[l],
   cnt = per-token counts of DIFFS==r (pure geometry).
 - attention = banded matmul: ApadT[c,l] = qrel[l, k(c-l)] scattered on a
   diagonal access pattern, attnT = vpad.T @ ApadT, plus rank-1 qmd x vsum.
 - 1/D is folded into qdot before building qrel/qmd -> numerator comes out
   pre-divided. Output bias bo added on host.
"""

import os
import sys

import numpy as np

sys.path.insert(0, "/opt/trn_rl_repo")

import concourse.bass as bass  # noqa: E402
import concourse.bacc as bacc  # noqa: E402
import concourse.tile as tile  # noqa: E402
from concourse import mybir  # noqa: E402

B, DIM, HEADS, DH = 8, 1024, 8, 128
MFEAT, MAXD, IMG = 621, 4, 10
L = IMG * IMG + 1  # 101
K = 2 * MAXD - 1  # 7
K2 = K * K  # 49
R = MAXD + 1  # 5
BC, HC = 4, 2  # batches / heads per core
NCORES = 8
BLC = BC * L  # 404 rows per core
DN = float(DH) ** -0.25
HALF_DN2 = 0.5 * float(DH) ** -0.5
EPS = 1e-4
AF = mybir.ActivationFunctionType
DT = mybir.dt.float32
AX = mybir.AxisListType
OP = mybir.AluOpType


def _geometry():
    m = MAXD
    base = np.arange(-m + 1, m)
    dx = np.broadcast_to(np.tile(base, K), (L, K * K))
    dy = np.broadcast_to(np.repeat(base, K), (L, K * K))
    x_off = np.concatenate([np.tile(np.arange(IMG), IMG), [0]])[:, None]
    y_off = np.concatenate([np.repeat(np.arange(IMG), IMG), [IMG]])[:, None]
    x_pos, y_pos = dx + x_off, dy + y_off
    diffs = np.abs(dx) + np.abs(dy)
    valid = (x_pos >= 0) & (x_pos < IMG) & (y_pos >= 0) & (y_pos < IMG) & (diffs < m)
    diffs = np.where(valid, diffs, m)
    masks = np.stack([(diffs == r).astype(np.float32) for r in range(4)])  # [4,L,K2]
    cnt5 = np.zeros((L, 5), np.float32)
    for r in range(4):
        cnt5[:, r] = (diffs == r).sum(1)
    cnt5[:, 4] = L
    return masks, cnt5


MASKS, CNT5 = _geometry()
# rfm5 = C5 @ rf'  (rows 0..3: rf'[r]-mde, row 4: mde);  lhsT = C5.T
C5 = np.zeros((5, 5), np.float32)
for _r in range(4):
    C5[_r, _r] = 1.0
    C5[_r, 4] = -1.0
C5[4, 4] = 1.0


def _emit(nc, tc, d):
    """Emit the per-core program. d: dict of dram tensor handles."""
    MT = [128, 128, 128, 128, 109]  # MFEAT partition tiles

    with (
        tc.tile_pool(name="const", bufs=1) as cpool,
        tc.tile_pool(name="head", bufs=2) as hpool,
        tc.tile_pool(name="work", bufs=2) as wpool,
        tc.tile_pool(name="outb", bufs=2) as opool,
        tc.tile_pool(name="ps_qv", bufs=2, space="PSUM") as ps_qv,
        tc.tile_pool(name="ps_dd", bufs=1, space="PSUM") as ps_dd,
        tc.tile_pool(name="ps_sm", bufs=1, space="PSUM") as ps_sm,
        tc.tile_pool(name="ps_tr", bufs=2, space="PSUM") as ps_tr,
        tc.tile_pool(name="ps_po", bufs=1, space="PSUM") as ps_po,
    ):
        # ---- constant loads ----
        xt = cpool.tile([128, 8 * BLC], DT)
        wq = cpool.tile([128, 8 * 256], DT)
        wv = cpool.tile([128, 8 * 256], DT)
        wo = cpool.tile([128, 2 * 1024], DT)
        for k in range(8):
            nc.sync.dma_start(xt[:, k * BLC:(k + 1) * BLC], d["xT"][k * 128:(k + 1) * 128, :])
            nc.sync.dma_start(wq[:, k * 256:(k + 1) * 256], d["wqT"][k * 128:(k + 1) * 128, :])
            nc.sync.dma_start(wv[:, k * 256:(k + 1) * 256], d["wvT"][k * 128:(k + 1) * 128, :])
        for k in range(2):
            nc.sync.dma_start(wo[:, k * 1024:(k + 1) * 1024], d["woT"][k * 128:(k + 1) * 128, :])
        projdn = cpool.tile([128, MFEAT], DT)
        nc.sync.dma_start(projdn[:], d["projdn"][:, :])
        rpeT = cpool.tile([128, 5], DT)
        nc.sync.dma_start(rpeT[:], d["rpeT"][:, :])
        c5T = cpool.tile([5, 5], DT)
        nc.sync.dma_start(c5T[:], d["c5T"][:, :])
        ident = cpool.tile([128, 128], DT)
        nc.sync.dma_start(ident[:], d["ident"][:, :])
        masks = cpool.tile([L, 4 * K2], DT)
        nc.sync.dma_start(masks[:], d["masks"][:, :])
        cnt5 = cpool.tile([L, 5], DT)
        nc.sync.dma_start(cnt5[:], d["cnt5"][:, :])
        bq = cpool.tile([128, 2], DT)
        nc.sync.dma_start(bq[:], d["bqh"][:, :])
        bv = cpool.tile([128, 2], DT)
        nc.sync.dma_start(bv[:], d["bvh"][:, :])
        halfdn2 = cpool.tile([128, 1], DT)
        nc.vector.memset(halfdn2[:], HALF_DN2)

        # persistent working buffers (manually ping-ponged; zeros static)
        apad = cpool.tile([L, 2 * 169], DT)  # banded matrix, 2 slots
        nc.gpsimd.memset(apad[:], 0.0)
        attnT = cpool.tile([128, 8 * L], DT)  # per (h,b) attention outputs
        rfm5T = cpool.tile([128, 25], DT)  # [mt,5] tiles of rfm5.T

        # ---- rf chain (tiny, once) ----
        rpeT2 = cpool.tile([128, 5], DT)
        nc.vector.tensor_mul(rpeT2[:], rpeT[:], rpeT[:])
        ddr = ps_dd.tile([5, MFEAT], DT, tag="dd")
        nc.tensor.matmul(ddr[:, 0:512], rpeT[:], projdn[:, 0:512], start=True, stop=True)
        nc.tensor.matmul(ddr[:, 512:MFEAT], rpeT[:], projdn[:, 512:MFEAT], start=True, stop=True)
        smr = ps_sm.tile([128, 512], DT, tag="sm")
        nc.tensor.matmul(smr[0:5, 0:1], rpeT2[:], halfdn2[:], start=True, stop=True)  # diag_r
        m5 = cpool.tile([5, 1], DT)
        nc.vector.reduce_max(m5[:], ddr[:], axis=AX.X)
        mx1 = cpool.tile([1, 1], DT)
        nc.gpsimd.tensor_reduce(mx1[:], m5[:], axis=AX.C, op=OP.max)
        ones5 = cpool.tile([1, 5], DT)
        nc.vector.memset(ones5[:], 1.0)
        nc.tensor.matmul(smr[0:5, 4:5], ones5[:], mx1[:], start=True, stop=True)  # MX bcast
        diagr = cpool.tile([5, 1], DT)
        nc.scalar.copy(diagr[:], smr[0:5, 0:1])
        nbr = cpool.tile([5, 1], DT)
        nc.vector.tensor_add(nbr[:], diagr[:], smr[0:5, 4:5])
        nc.vector.tensor_scalar_mul(nbr[:], nbr[:], -1.0)
        rfp = cpool.tile([5, MFEAT], DT)
        nc.scalar.activation(rfp[:], ddr[:], AF.Exp, bias=nbr[:], scale=1.0)
        nc.vector.tensor_scalar_add(rfp[:], rfp[:], EPS)
        rfm5p = ps_dd.tile([5, MFEAT], DT, tag="dd")
        nc.tensor.matmul(rfm5p[:, 0:512], c5T[:], rfp[:, 0:512], start=True, stop=True)
        nc.tensor.matmul(rfm5p[:, 512:MFEAT], c5T[:], rfp[:, 512:MFEAT], start=True, stop=True)
        rfm5 = cpool.tile([5, MFEAT], DT)
        nc.scalar.copy(rfm5[:], rfm5p[:])
        for t in range(5):
            mt = MT[t]
            trp = ps_tr.tile([128, 128], DT, tag="tr")
            nc.tensor.transpose(trp[0:mt, 0:5], rfm5[:, t * 128:t * 128 + mt], ident[0:5, 0:5])
            nc.scalar.copy(rfm5T[0:mt, t * 5:(t + 1) * 5], trp[0:mt, 0:5])

        apitch = int(apad.tensor.shape[-1])  # allocated row pitch for diagonal AP

        # ---- main loop: heads x batches ----
        for h in range(HC):
            qTp = ps_qv.tile([128, BLC], DT, tag="qv")
            for k in range(8):
                nc.tensor.matmul(
                    qTp[:], wq[:, k * 256 + h * 128:k * 256 + (h + 1) * 128],
                    xt[:, k * BLC:(k + 1) * BLC], start=(k == 0), stop=(k == 7))
            qT = hpool.tile([128, BLC], DT, tag="qT")
            nc.scalar.activation(qT[:], qTp[:], AF.Identity, bias=bq[:, h:h + 1], scale=1.0)
            vTp = ps_qv.tile([128, BLC], DT, tag="qv")
            for k in range(8):
                nc.tensor.matmul(
                    vTp[:], wv[:, k * 256 + h * 128:k * 256 + (h + 1) * 128],
                    xt[:, k * BLC:(k + 1) * BLC], start=(k == 0), stop=(k == 7))
            vT = hpool.tile([128, BLC], DT, tag="vT")
            nc.scalar.activation(vT[:], vTp[:], AF.Identity, bias=bv[:, h:h + 1], scale=1.0)
            qT2 = hpool.tile([128, BLC], DT, tag="qT2")
            nc.vector.tensor_mul(qT2[:], qT[:], qT[:])

            for b in range(BC):
                pair = h * BC + b
                slot = pair % 2
                bl = slice(b * L, (b + 1) * L)

                # dd + diag + rowmax -> exp -> qf'
                dd = ps_dd.tile([L, MFEAT], DT, tag="dd")
                nc.tensor.matmul(dd[:, 0:512], qT[:, bl], projdn[:, 0:512], start=True, stop=True)
                nc.tensor.matmul(dd[:, 512:MFEAT], qT[:, bl], projdn[:, 512:MFEAT], start=True, stop=True)
                sm = ps_sm.tile([128, 512], DT, tag="sm")
                nc.tensor.matmul(sm[0:L, 0:1], qT2[:, bl], halfdn2[:], start=True, stop=True)
                mxn = wpool.tile([L, 1], DT, tag="mxn")
                nc.vector.reduce_max(mxn[:], dd[:], axis=AX.X, negate=True)
                nb = wpool.tile([L, 1], DT, tag="nb")
                nc.vector.tensor_sub(nb[:], mxn[:], sm[0:L, 0:1])
                qf = wpool.tile([L, MFEAT], DT, tag="qf")
                nc.scalar.activation(qf[:], dd[:], AF.Exp, bias=nb[:], scale=1.0)
                nc.vector.tensor_scalar_add(qf[:], qf[:], EPS)

                # transpose qf -> qfT tiles; qdot5 = qfT.T @ rfm5T
                qfT = wpool.tile([128, 5 * L], DT, tag="qfT")
                for t in range(5):
                    mt = MT[t]
                    trp = ps_tr.tile([128, 128], DT, tag="tr")
                    nc.tensor.transpose(trp[0:mt, 0:L], qf[:, t * 128:t * 128 + mt], ident[0:L, 0:L])
                    nc.scalar.copy(qfT[0:mt, t * L:(t + 1) * L], trp[0:mt, 0:L])
                qd5p = ps_sm.tile([128, 512], DT, tag="sm")
                for t in range(5):
                    mt = MT[t]
                    nc.tensor.matmul(
                        qd5p[0:L, 8:13], qfT[0:mt, t * L:(t + 1) * L],
                        rfm5T[0:mt, t * 5:(t + 1) * 5], start=(t == 0), stop=(t == 4))

                # D_inv, scaled qdot
                dm = wpool.tile([L, 5], DT, tag="dm")
                nc.vector.tensor_mul(dm[:], qd5p[0:L, 8:13], cnt5[:])
                draw = wpool.tile([L, 1], DT, tag="draw")
                nc.vector.reduce_sum(draw[:], dm[:], axis=AX.X)
                dinv = wpool.tile([L, 1], DT, tag="dinv")
                nc.vector.reciprocal(dinv[:], draw[:])
                qd5s = wpool.tile([L, 5], DT, tag="qd5s")
                nc.vector.tensor_scalar_mul(qd5s[:], qd5p[0:L, 8:13], dinv[:])

                # qrel = sum_r qd5s[:,r] * MASK_r
                t0 = wpool.tile([L, K2], DT, tag="t0")
                t1 = wpool.tile([L, K2], DT, tag="t1")
                nc.scalar.mul(t0[:], masks[:, 0:K2], qd5s[:, 0:1])
                nc.scalar.mul(t1[:], masks[:, K2:2 * K2], qd5s[:, 1:2])
                nc.vector.tensor_add(t0[:], t0[:], t1[:])
                t2 = wpool.tile([L, K2], DT, tag="t2")
                t3 = wpool.tile([L, K2], DT, tag="t3")
                nc.scalar.mul(t2[:], masks[:, 2 * K2:3 * K2], qd5s[:, 2:3])
                nc.scalar.mul(t3[:], masks[:, 3 * K2:4 * K2], qd5s[:, 3:4])
                nc.vector.tensor_add(t2[:], t2[:], t3[:])
                qrel = wpool.tile([L, K2], DT, tag="qrel")
                nc.vector.tensor_add(qrel[:], t0[:], t2[:])

                # diagonal scatter: apad[l, 33 + l + 10j + i] = qrel[l, 7j+i]
                # apad[l, l + 10j + i] = qrel[l, 7j+i]; band cols 33..133 are read
                dst = bass.AP(apad.tensor, slot * 169,
                              [[apitch + 1, L], [10, 7], [1, 7]])
                src = qrel[:].rearrange("l (j i) -> l j i", j=7)
                nc.sync.dma_start(dst, src)

                # v in [token, d] layout via PE transpose of vT slice
                vtrp = ps_tr.tile([128, 128], DT, tag="tr")
                nc.tensor.transpose(vtrp[0:L, 0:128], vT[:, bl], ident[:, :])
                v_td = wpool.tile([L, 128], DT, tag="v_td")
                nc.scalar.copy(v_td[:], vtrp[0:L, 0:128])

                # vsum row + qmd row
                vsc = wpool.tile([128, 1], DT, tag="vsc")
                nc.vector.reduce_sum(vsc[:], vT[:, bl], axis=AX.X)
                vsr_p = ps_sm.tile([128, 512], DT, tag="sm")
                nc.tensor.transpose(vsr_p[0:1, 16:144], vsc[:], ident[0:128, 0:128])
                vsr = wpool.tile([1, 128], DT, tag="vsr")
                nc.scalar.copy(vsr[:], vsr_p[0:1, 16:144])
                qmd_p = ps_sm.tile([128, 512], DT, tag="sm")
                nc.tensor.transpose(qmd_p[0:1, 150:251], qd5s[:, 4:5], ident[0:L, 0:L])
                qmd = wpool.tile([1, L], DT, tag="qmd")
                nc.scalar.copy(qmd[:], qmd_p[0:1, 150:251])

                # ApadT middle band (rows 33..133 are the only nonzero reads)
                a0p = ps_tr.tile([128, 128], DT, tag="tr")
                nc.tensor.transpose(a0p[0:L, 0:L], apad[:, slot * 169 + 33:slot * 169 + 134], ident[0:L, 0:L])
                a0 = wpool.tile([L, L], DT, tag="a0")
                nc.scalar.copy(a0[:], a0p[0:L, 0:L])

                # attnT = v_td.T @ band + vsum x qmd   [128(d), L]
                atp = ps_tr.tile([128, 128], DT, tag="tr")
                nc.tensor.matmul(atp[0:128, 0:L], v_td[:], a0[:], start=True, stop=False)
                nc.tensor.matmul(atp[0:128, 0:L], vsr[:], qmd[:], start=False, stop=True)
                nc.scalar.copy(attnT[:, pair * L:(pair + 1) * L], atp[0:128, 0:L])

        # ---- output projection: per batch, sum over heads ----
        for b in range(BC):
            for n in range(2):
                po = ps_po.tile([L, 512], DT, tag="po")
                for h in range(HC):
                    pair = h * BC + b
                    nc.tensor.matmul(
                        po[:], attnT[:, pair * L:(pair + 1) * L],
                        wo[:, h * 1024 + n * 512:h * 1024 + (n + 1) * 512],
                        start=(h == 0), stop=(h == HC - 1))
                ob = opool.tile([L, 512], DT, tag="ob")
                nc.vector.tensor_copy(ob[:], po[:])
                nc.sync.dma_start(d["out_p"][b * L:(b + 1) * L, n * 512:(n + 1) * 512], ob[:])


_CACHE = {}


def _build():
    if "nc" in _CACHE:
        return _CACHE["nc"]
    nc = bacc.Bacc("TRN2", target_bir_lowering=False, debug=False)
    d = {
        "xT": nc.dram_tensor("xT", [DIM, BLC], DT, kind="ExternalInput"),
        "wqT": nc.dram_tensor("wqT", [DIM, 256], DT, kind="ExternalInput"),
        "wvT": nc.dram_tensor("wvT", [DIM, 256], DT, kind="ExternalInput"),
        "woT": nc.dram_tensor("woT", [256, 1024], DT, kind="ExternalInput"),
        "projdn": nc.dram_tensor("projdn", [128, MFEAT], DT, kind="ExternalInput"),
        "rpeT": nc.dram_tensor("rpeT", [128, 5], DT, kind="ExternalInput"),
        "c5T": nc.dram_tensor("c5T", [5, 5], DT, kind="ExternalInput"),
        "ident": nc.dram_tensor("ident", [128, 128], DT, kind="ExternalInput"),
        "masks": nc.dram_tensor("masks", [L, 4 * K2], DT, kind="ExternalInput"),
        "cnt5": nc.dram_tensor("cnt5", [L, 5], DT, kind="ExternalInput"),
        "bqh": nc.dram_tensor("bqh", [128, 2], DT, kind="ExternalInput"),
        "bvh": nc.dram_tensor("bvh", [128, 2], DT, kind="ExternalInput"),
        "out_p": nc.dram_tensor("out_p", [BLC, 1024], DT, kind="ExternalOutput"),
    }
    with tile.TileContext(nc) as tc:
        _emit(nc, tc, d)
    nc.compile()
    _CACHE["nc"] = nc
    return nc


def make_in_maps(x, Wq, bq, Wv, bv, rpe, proj, Wo, bo):
    f = np.float32
    x = np.asarray(x, f)
    ident = np.eye(128, dtype=f)
    projdn = np.ascontiguousarray(proj.T.astype(f) * np.float32(DN))
    rpeT = np.ascontiguousarray(np.asarray(rpe, f).T)
    masks_h = np.ascontiguousarray(np.concatenate([MASKS[r] for r in range(4)], axis=1))
    in_maps = []
    for c in range(NCORES):
        bg, hg = c // 4, c % 4
        h0 = hg * HC
        cols = slice(h0 * DH, (h0 + HC) * DH)
        xc = x[bg * BC:(bg + 1) * BC].reshape(BLC, DIM)
        in_maps.append({
            "xT": np.ascontiguousarray(xc.T),
            "wqT": np.ascontiguousarray(np.asarray(Wq, f)[cols, :].T),
            "wvT": np.ascontiguousarray(np.asarray(Wv, f)[cols, :].T),
            "woT": np.ascontiguousarray(np.asarray(Wo, f)[:, cols].T),
            "projdn": projdn,
            "rpeT": rpeT,
            "c5T": np.ascontiguousarray(C5.T),
            "ident": ident,
            "masks": masks_h,
            "cnt5": CNT5,
            "bqh": np.ascontiguousarray(np.asarray(bq, f)[cols].reshape(HC, DH).T),
            "bvh": np.ascontiguousarray(np.asarray(bv, f)[cols].reshape(HC, DH).T),
        })
    return in_maps


def kernel(x, Wq, bq, Wv, bv, rpe, proj, Wo, bo):
    from concourse.bass_utils import run_bass_kernel_spmd

    nc = _build()
    in_maps = make_in_maps(x, Wq, bq, Wv, bv, rpe, proj, Wo, bo)
    trace = bool(int(os.environ.get("TRNK_TRACE", "0")))
    res = run_bass_kernel_spmd(nc, in_maps, core_ids=list(range(NCORES)), trace=trace)
    if trace and res.exec_time_ns is not None:
        print(f"HW exec time: {res.exec_time_ns} ns")
    out = np.zeros((B, L, DIM), np.float32)
    for c in range(NCORES):
        bg = c // 4
        out[bg * BC:(bg + 1) * BC] += res.results[c]["out_p"].reshape(BC, L, DIM)
    out += np.asarray(bo, np.float32)
    return out
